# revision 19
# baseline (speedup 1.0000x reference)
"""AttnBlock (GroupNorm + single-head self-attention + residual) on 8 TRN2 cores.

Problem: x [4, 512, 64, 64] f32.  out = x + proj_out(attn(GN(x))) with
1x1-conv projections and softmax attention over the 4096 spatial positions.

Sharding: 8 cores = 4 batch elements x 2 query-halves.  Each core gets the
full (column-rotated) [512, 4096] slab of its batch element, computes
GroupNorm + K/V projections over all 4096 positions, and Q/attention/output
for its 2048 query columns.  Column rotation makes the program SPMD: every
core's queries are columns 0:2048 of its own input.  No collectives; the
host gathers the 8 [512, 2048] results.

All heavy matmuls run as float32r (TF32-like, full PE rate at N=512,
~1.5e-4 matmul rel-err).  GroupNorm stats and softmax accumulate in f32.
Softmax: scores^T tiles [keys,128 x q,512] so QK^T and AV need no
transposes; row sums via ones-vector matmul; no max subtraction (logits are
~N(0,1), |logit| << 80).
"""
import numpy as np

import concourse.bacc as bacc
import concourse.tile as tile
import concourse.mybir as mybir
from concourse.bass_utils import run_bass_kernel_spmd

F32 = mybir.dt.float32
F32R = mybir.dt.float32r
AF = mybir.ActivationFunctionType
ALU = mybir.AluOpType

B, C, H, W = 4, 512, 64, 64
N = H * W           # 4096 key positions
NQ = N // 2         # 2048 query positions per core
P = 128             # partitions
CCH = C // P        # 4 channel chunks
QS = 512            # query-chunk width
NQCH = NQ // QS     # 4 query chunks
MCH = N // P        # 32 key tiles
NSUB = 8            # bn_stats subtiles (4096 / 512)
NUM_GROUPS = 32
GPC = P // (C // NUM_GROUPS)   # groups per 128-channel chunk = 8
EPS = 1e-6


OPTS = {"stagger": True, "defer": True, "act_norm": True, "psq_share": True, "split_kv": True, "kcopy_dve": False, "qfuse": True, "ps2_pso": True, "ea4": True, "fold": True}


def build_nc(reps: int = 1, opts=None):
    o_ = dict(OPTS)
    if opts:
        o_.update(opts)
    nc = bacc.Bacc()
    x_d = nc.dram_tensor("x", [C, N], F32, kind="ExternalInput")
    wqt_d = nc.dram_tensor("wqt", [C, C], F32, kind="ExternalInput")
    wkt_d = nc.dram_tensor("wkt", [C, C], F32, kind="ExternalInput")
    wvt_d = nc.dram_tensor("wvt", [C, C], F32, kind="ExternalInput")
    wot_d = nc.dram_tensor("wot", [C, C], F32, kind="ExternalInput")
    bq_d = nc.dram_tensor("bq", [C], F32, kind="ExternalInput")
    bk_d = nc.dram_tensor("bk", [C], F32, kind="ExternalInput")
    bvrow_d = nc.dram_tensor("bvrow", [1, C], F32, kind="ExternalInput")
    gamma_d = nc.dram_tensor("gamma", [C], F32, kind="ExternalInput")
    beta_d = nc.dram_tensor("beta", [C], F32, kind="ExternalInput")
    xr_d = nc.dram_tensor("xr", [C, NQ], F32, kind="ExternalInput")
    gmask_d = nc.dram_tensor("gmask", [P, GPC], F32, kind="ExternalInput")
    gbcast_d = nc.dram_tensor("gbcast", [GPC, P], F32, kind="ExternalInput")
    ones128_d = nc.dram_tensor("ones128", [P, 1], F32, kind="ExternalInput")
    y_d = nc.dram_tensor("y", [C, NQ], F32, kind="ExternalOutput")

    x_t = x_d.rearrange("(c p) n -> p c n", p=P)
    xr_t = xr_d.rearrange("(c p) n -> p c n", p=P)
    y_t = y_d.rearrange("(c p) n -> p c n", p=P)

    with tile.TileContext(nc) as tc:
        def body(_iv=None):
            with tc.tile_pool(name="consts", bufs=1) as consts, \
                 tc.tile_pool(name="dram", bufs=1, space="DRAM") as dpool:
                hd = None
                if not o_["fold"]:
                    hd = dpool.tile([P, CCH, N], F32R, tag="hd")

                gam = consts.tile([P, CCH], F32, tag="gam")
                bet = consts.tile([P, CCH], F32, tag="bet")
                gmask = consts.tile([P, GPC], F32, tag="gmask")
                gbcast = consts.tile([GPC, P], F32, tag="gbcast")
                ones128 = consts.tile([P, 1], F32R, tag="ones128")
                bvrow = consts.tile([1, C], F32, tag="bvrow")
                bvb = consts.tile([P, C], F32, tag="bvb")
                bqs = consts.tile([P, CCH], F32, tag="bqs")
                bks = consts.tile([P, CCH], F32, tag="bks")
                epst = consts.tile([GPC, 1], F32, tag="epst")
                nc.sync.dma_start(out=gam, in_=gamma_d.rearrange("(c p) -> p c", p=P))
                nc.sync.dma_start(out=bet, in_=beta_d.rearrange("(c p) -> p c", p=P))
                nc.sync.dma_start(out=gmask, in_=gmask_d[:, :])
                nc.sync.dma_start(out=gbcast, in_=gbcast_d[:, :])
                nc.gpsimd.dma_start(out=ones128, in_=ones128_d[:, :])
                nc.sync.dma_start(out=bvrow, in_=bvrow_d[:, :])
                if not o_["fold"]:
                    nc.gpsimd.partition_broadcast(bvb, bvrow)
                nc.sync.dma_start(out=bqs, in_=bq_d.rearrange("(c p) -> p c", p=P))
                nc.sync.dma_start(out=bks, in_=bk_d.rearrange("(c p) -> p c", p=P))
                nc.vector.memset(epst, EPS)

                # ---------------- Phase A: GroupNorm -> h (DRAM, f32r) ----
                with tc.tile_pool(name="pA", bufs=1) as pA, \
                     tc.tile_pool(name="pAs", bufs=2) as pAs, \
                     tc.tile_pool(name="pAh", bufs=3) as pAh, \
                     tc.tile_pool(name="psA", bufs=2, space="PSUM") as psA:
                    xt = pA.tile([P, CCH, N], F32, tag="xt")
                    for c in range(CCH):
                        for s in range(NSUB):
                            nc.sync.dma_start(
                                out=xt[:, c, s * QS:(s + 1) * QS],
                                in_=x_t[:, c, s * QS:(s + 1) * QS])

                    stats = pA.tile([P, CCH, NSUB, 6], F32, tag="stats")
                    mv = pA.tile([P, CCH, 2], F32, tag="mv")
                    st3 = pA.tile([P, CCH, 3], F32, tag="st3")
                    scb = consts.tile([P, CCH, 2], F32, tag="scb")
                    for c in range(CCH):
                        for s in range(NSUB):
                            nc.vector.bn_stats(
                                out=stats[:, c, s, :],
                                in_=xt[:, c, s * QS:(s + 1) * QS])
                        nc.vector.bn_aggr(out=mv[:, c, :], in_=stats[:, c, :, :])
                        nc.vector.tensor_copy(st3[:, c, 0:2], mv[:, c, :])
                        nc.vector.tensor_mul(
                            st3[:, c, 2:3], mv[:, c, 0:1], mv[:, c, 0:1])
                    for c in range(CCH):
                        psg = psA.tile([GPC, 3], F32, tag="psg")
                        nc.tensor.matmul(psg, gmask, st3[:, c, :],
                                         start=True, stop=True)
                        sg = pAs.tile([GPC, 3], F32, tag="sg")
                        nc.scalar.activation(sg, psg, AF.Copy)
                        gst = pAs.tile([GPC, 4], F32, tag="gst")
                        # var_g = E[var] + E[mean^2] - (E[mean])^2
                        nc.vector.tensor_add(gst[:, 1:2], sg[:, 1:2], sg[:, 2:3])
                        nc.vector.tensor_mul(gst[:, 2:3], sg[:, 0:1], sg[:, 0:1])
                        nc.vector.tensor_tensor(
                            out=gst[:, 1:2], in0=gst[:, 1:2], in1=gst[:, 2:3],
                            op=ALU.subtract)
                        nc.scalar.activation(gst[:, 3:4], gst[:, 1:2], AF.Sqrt,
                                             bias=epst)
                        nc.vector.reciprocal(gst[:, 3:4], gst[:, 3:4])
                        gs2 = pAs.tile([GPC, 2], F32, tag="gs2")
                        nc.vector.tensor_copy(gs2[:, 0:1], sg[:, 0:1])
                        nc.vector.tensor_copy(gs2[:, 1:2], gst[:, 3:4])
                        psb = psA.tile([P, 2], F32, tag="psb")
                        nc.tensor.matmul(psb, gbcast, gs2, start=True, stop=True)
                        # scale_c = gamma*rstd ; bias_c = beta - mean*scale
                        nc.vector.tensor_mul(
                            scb[:, c, 0:1], gam[:, c:c + 1], psb[:, 1:2])
                        tmp = pAs.tile([P, 1], F32, tag="tmp")
                        nc.vector.tensor_mul(tmp, psb[:, 0:1], scb[:, c, 0:1])
                        nc.vector.tensor_tensor(
                            out=scb[:, c, 1:2], in0=bet[:, c:c + 1], in1=tmp,
                            op=ALU.subtract)
                    if o_["fold"]:
                        bias_r = consts.tile([P, CCH, 4], F32R, tag="bias_r")
                        for j in range(4):
                            nc.vector.tensor_copy(bias_r[:, :, j], scb[:, :, 1])
                    for s in range(NSUB if not o_["fold"] else 0):
                        for c in range(CCH):
                            ht = pAh.tile([P, QS], F32R, tag="ht")
                            if o_["act_norm"]:
                                nc.scalar.activation(
                                    ht, xt[:, c, s * QS:(s + 1) * QS],
                                    AF.Identity,
                                    bias=scb[:, c, 1:2], scale=scb[:, c, 0:1])
                            else:
                                nc.vector.tensor_scalar(
                                    out=ht, in0=xt[:, c, s * QS:(s + 1) * QS],
                                    scalar1=scb[:, c, 0:1],
                                    scalar2=scb[:, c, 1:2],
                                    op0=ALU.mult, op1=ALU.add)
                            nc.sync.dma_start(
                                out=hd[:, c, s * QS:(s + 1) * QS], in_=ht)

                # ---------------- persistent K / V^T ----------------------
                with tc.tile_pool(name="pKV", bufs=1) as pKV:
                    if o_["qfuse"]:
                        qts = [pKV.tile([P, CCH, QS], F32R, tag=f"qts{_s}",
                                        name=f"qts{_s}")
                               for _s in range(NQCH)]
                    if o_["split_kv"]:
                        ks = [pKV.tile([P, CCH, QS], F32R, tag=f"ks{_s}",
                                       name=f"ks{_s}")
                              for _s in range(NSUB)]
                        vts = [pKV.tile([P, CCH, C], F32R, tag=f"vts{_s}",
                                        name=f"vts{_s}")
                               for _s in range(NSUB)]
                    else:
                        k_one = pKV.tile([P, CCH, N], F32R, tag="k_sb")
                        vt_one = pKV.tile([P, NSUB, CCH, C], F32R,
                                          tag="vt_sb")
                        ks = [k_one[:, :, _s * QS:(_s + 1) * QS]
                              for _s in range(NSUB)]
                        vts = [vt_one[:, _s] for _s in range(NSUB)]

                    # -------- Phase B: K and V^T projections --------------
                    with tc.tile_pool(name="pBw", bufs=1) as pBw, \
                         tc.tile_pool(name="pBh",
                                      bufs=2 if o_["qfuse"] else 3) as pBh, \
                         tc.tile_pool(name="psB",
                                      bufs=2 if o_["qfuse"] else 4,
                                      space="PSUM") as psB:
                        wkt = pBw.tile([P, CCH, C], F32R, tag="wkt")
                        wvt = pBw.tile([P, CCH, C], F32R, tag="wvt")
                        nc.gpsimd.dma_start(
                            out=wkt, in_=wkt_d.rearrange("(c p) o -> p c o", p=P))
                        nc.gpsimd.dma_start(
                            out=wvt, in_=wvt_d.rearrange("(c p) o -> p c o", p=P))
                        if o_["qfuse"]:
                            wqt_b = pBw.tile([P, CCH, C], F32R, tag="wqt_b")
                            nc.gpsimd.dma_start(
                                out=wqt_b,
                                in_=wqt_d.rearrange("(c p) o -> p c o", p=P))
                        biask = bks
                        biasq = bqs
                        if o_["fold"]:
                            # h = S.x + b folds into the projections:
                            #   k = (wk.S).x + (wk.b + bk)  etc.
                            # bias matvecs first (need unscaled weights)
                            with tc.tile_pool(name="psBb", bufs=1,
                                              space="PSUM") as psBb:
                                kqb = pBw.tile([P, CCH, 2], F32, tag="kqb")
                                for o in range(CCH):
                                    for j, wt in enumerate((wkt, wqt_b)):
                                        psbb = psBb.tile(
                                            [P, 4], F32, tag="psbb",
                                            name=f"psbb{o}_{j}")
                                        for c in range(CCH):
                                            nc.tensor.matmul(
                                                psbb,
                                                wt[:, c, o * P:(o + 1) * P],
                                                bias_r[:, c, :],
                                                start=(c == 0),
                                                stop=(c == CCH - 1))
                                        nc.scalar.activation(
                                            kqb[:, o, j:j + 1],
                                            psbb[:, 0:1], AF.Copy)
                                psvb = psBb.tile([1, C], F32, tag="psvb")
                                for c in range(CCH):
                                    nc.tensor.matmul(
                                        psvb, bias_r[:, c, 0:1], wvt[:, c, :],
                                        start=(c == 0), stop=(c == CCH - 1))
                                vbrow = pBw.tile([1, C], F32, tag="vbrow")
                                nc.vector.tensor_add(vbrow, psvb, bvrow)
                                nc.gpsimd.partition_broadcast(bvb, vbrow)
                            biask = pBw.tile([P, CCH], F32, tag="biask")
                            biasq = pBw.tile([P, CCH], F32, tag="biasq")
                            nc.vector.tensor_add(biask, kqb[:, :, 0], bks)
                            nc.vector.tensor_add(biasq, kqb[:, :, 1], bqs)
                            # scale weight rows in place (f32r -> f32r)
                            for c in range(CCH):
                                for wt in (wkt, wvt, wqt_b):
                                    nc.vector.tensor_scalar_mul(
                                        out=wt[:, c, :], in0=wt[:, c, :],
                                        scalar1=scb[:, c, 0:1])
                        for s in range(NSUB):
                            ht = pBh.tile([P, CCH, QS], F32R, tag="htb")
                            if o_["fold"]:
                                nc.gpsimd.dma_start(
                                    out=ht,
                                    in_=x_t[:, :, s * QS:(s + 1) * QS])
                            else:
                                nc.sync.dma_start(
                                    out=ht,
                                    in_=hd[:, :, s * QS:(s + 1) * QS])
                            for o in range(CCH):
                                psk = psB.tile([P, QS], F32, tag="psk")
                                for c in range(CCH):
                                    nc.tensor.matmul(
                                        psk, wkt[:, c, o * P:(o + 1) * P],
                                        ht[:, c, :],
                                        start=(c == 0), stop=(c == CCH - 1))
                                if o_["kcopy_dve"]:
                                    nc.vector.tensor_scalar(
                                        out=ks[s][:, o, :], in0=psk,
                                        scalar1=biask[:, o:o + 1],
                                        scalar2=None, op0=ALU.add)
                                else:
                                    nc.scalar.activation(
                                        ks[s][:, o, :], psk,
                                        AF.Identity, bias=biask[:, o:o + 1])
                            for mm in range(CCH):
                                m = s * CCH + mm
                                psv = psB.tile([P, C], F32, tag="psv")
                                for c in range(CCH):
                                    nc.tensor.matmul(
                                        psv, ht[:, c, mm * P:(mm + 1) * P],
                                        wvt[:, c, :],
                                        start=(c == 0), stop=(c == CCH - 1))
                                nc.vector.tensor_add(vts[s][:, mm, :], psv,
                                                     bvb)
                            if o_["qfuse"] and s < NQCH:
                                for o in range(CCH):
                                    psq = psB.tile([P, QS], F32, tag="psq",
                                                   name=f"psqB{s}_{o}")
                                    for c in range(CCH):
                                        nc.tensor.matmul(
                                            psq,
                                            wqt_b[:, c, o * P:(o + 1) * P],
                                            ht[:, c, :],
                                            start=(c == 0),
                                            stop=(c == CCH - 1))
                                    nc.scalar.activation(
                                        qts[s][:, o, :], psq, AF.Identity,
                                        bias=biasq[:, o:o + 1])

                    # -------- Phase C: Q, attention, output ---------------
                    with tc.tile_pool(name="pCw", bufs=1) as pCw, \
                         tc.tile_pool(name="pCh", bufs=1) as pCh, \
                         tc.tile_pool(name="pCq", bufs=2) as pCq, \
                         tc.tile_pool(name="pCo", bufs=1) as pCo, \
                         tc.tile_pool(name="pCe",
                                      bufs=4 if o_["ea4"] else 3) as pCe, \
                         tc.tile_pool(name="pCs", bufs=1) as pCs, \
                         tc.tile_pool(name="pCy", bufs=2) as pCy, \
                         tc.tile_pool(name="psO", bufs=4, space="PSUM") as psO_p, \
                         tc.tile_pool(name="psS", bufs=1, space="PSUM") as psS_p, \
                         tc.tile_pool(name="psAtt",
                                      bufs=3 if o_["psq_share"] else 2,
                                      space="PSUM") as psAtt, \
                         tc.tile_pool(name="psMM", bufs=1,
                                      space="PSUM") as psMM:
                        wot = pCw.tile([P, CCH, C], F32R, tag="wot")
                        nc.gpsimd.dma_start(
                            out=wot, in_=wot_d.rearrange("(c p) o -> p c o", p=P))
                        if not o_["qfuse"]:
                            wqt = pCw.tile([P, CCH, C], F32R, tag="wqt")
                            nc.gpsimd.dma_start(
                                out=wqt,
                                in_=wqt_d.rearrange("(c p) o -> p c o", p=P))

                        for iq in range(NQCH):
                            qsl = slice(iq * QS, (iq + 1) * QS)
                            if o_["qfuse"]:
                                qt = qts[iq]
                            else:
                                hq = pCh.tile([P, CCH, QS], F32R, tag="hq")
                                nc.sync.dma_start(out=hq, in_=hd[:, :, qsl])
                                qt = pCq.tile([P, CCH, QS], F32R, tag="qt")
                            for o in range(CCH if not o_["qfuse"] else 0):
                                if o_["psq_share"]:
                                    psq = psAtt.tile([P, QS], F32, tag="psa",
                                                     name=f"psq{o}")
                                else:
                                    psq = psMM.tile([P, QS], F32, tag="psmm",
                                                    name=f"psq{o}")
                                for c in range(CCH):
                                    nc.tensor.matmul(
                                        psq, wqt[:, c, o * P:(o + 1) * P],
                                        hq[:, c, :],
                                        start=(c == 0), stop=(c == CCH - 1))
                                if o_["kcopy_dve"]:
                                    nc.vector.tensor_scalar(
                                        out=qt[:, o, :], in0=psq,
                                        scalar1=bqs[:, o:o + 1], scalar2=None,
                                        op0=ALU.add)
                                else:
                                    nc.scalar.activation(
                                        qt[:, o, :], psq, AF.Identity,
                                        bias=bqs[:, o:o + 1])

                            psO = [psO_p.tile([P, QS], F32, tag="psO",
                                              name=f"psO{_c}")
                                   for _c in range(CCH)]
                            pssum = psS_p.tile([1, QS], F32, tag="pssum")
                            # software-pipelined: QK(m) runs ahead of AV(m-1)
                            # so the PE never waits on exp(m-1) (ACT).
                            ea_prev = None

                            def qk(m):
                                psa = psAtt.tile([P, QS], F32, tag="psa",
                                                 name=f"psa{m}")
                                for c in range(CCH):
                                    nc.tensor.matmul(
                                        psa,
                                        ks[m // CCH][:, c,
                                                     (m % CCH) * P:
                                                     (m % CCH + 1) * P],
                                        qt[:, c, :],
                                        start=(c == 0), stop=(c == CCH - 1))
                                ea = pCe.tile([P, QS], F32R, tag="ea",
                                              name=f"ea{m}")
                                nc.scalar.activation(ea, psa, AF.Exp)
                                return ea

                            def av(m, ea):
                                for c in range(CCH):
                                    nc.tensor.matmul(
                                        psO[c],
                                        vts[m // CCH][:, m % CCH,
                                                      c * P:(c + 1) * P], ea,
                                        start=(m == 0), stop=(m == MCH - 1),
                                        skip_group_check=True)
                                nc.tensor.matmul(
                                    pssum, ones128, ea,
                                    start=(m == 0), stop=(m == MCH - 1),
                                    skip_group_check=True)

                            if o_["stagger"]:
                                ea_prev = qk(0)
                                for m in range(1, MCH):
                                    ea_cur = qk(m)
                                    av(m - 1, ea_prev)
                                    ea_prev = ea_cur
                                av(MCH - 1, ea_prev)
                            else:
                                for m in range(MCH):
                                    av(m, qk(m))

                            rec = pCs.tile([1, QS], F32, tag="rec")
                            nc.vector.reciprocal(rec, pssum)
                            recb = pCs.tile([P, QS], F32, tag="recb")
                            nc.gpsimd.partition_broadcast(recb, rec)
                            on = pCo.tile([P, CCH, QS], F32R, tag="on")
                            if o_["defer"]:
                                # unnormalized O^T -> SBUF; out2 starts
                                # immediately; 1/rowsum applied at the end.
                                for c in range(CCH):
                                    nc.scalar.activation(on[:, c, :], psO[c],
                                                         AF.Copy)
                            else:
                                for c in range(CCH):
                                    nc.vector.tensor_mul(on[:, c, :], psO[c],
                                                         recb)
                            xrt = pCo.tile([P, CCH, QS], F32, tag="xrt")
                            nc.sync.dma_start(out=xrt, in_=xr_t[:, :, qsl])
                            for o in range(CCH):
                                if o_["ps2_pso"]:
                                    ps2 = psO_p.tile([P, QS], F32, tag="psO",
                                                     name=f"ps2{o}")
                                elif o_["psq_share"]:
                                    ps2 = psAtt.tile([P, QS], F32, tag="psa",
                                                     name=f"ps2{o}")
                                else:
                                    ps2 = psMM.tile([P, QS], F32, tag="psmm",
                                                    name=f"ps2{o}")
                                for c in range(CCH):
                                    nc.tensor.matmul(
                                        ps2, wot[:, c, o * P:(o + 1) * P],
                                        on[:, c, :],
                                        start=(c == 0), stop=(c == CCH - 1))
                                yt = pCy.tile([P, QS], F32, tag="yt")
                                if o_["defer"]:
                                    t1 = pCs.tile([P, QS], F32, tag="t1")
                                    nc.vector.tensor_mul(t1, ps2, recb)
                                    nc.vector.tensor_add(yt, t1, xrt[:, o, :])
                                else:
                                    nc.vector.tensor_add(yt, ps2, xrt[:, o, :])
                                nc.sync.dma_start(out=y_t[:, o, qsl], in_=yt)

        if reps == 1:
            body()
        else:
            with tc.For_i(0, reps, 1) as iv:
                body(iv)
    nc.compile()
    return nc


def _host_inputs(x, gn_gamma, gn_beta, wq, bq, wk, bk, wv, bv, wo, bo):
    """Build the 8 per-core input maps from full inputs."""
    s = 1.0 / np.sqrt(np.float32(C))
    shared = {
        "wqt": np.ascontiguousarray((wq * s).T.astype(np.float32)),
        "wkt": np.ascontiguousarray(wk.T.astype(np.float32)),
        "wvt": np.ascontiguousarray(wv.T.astype(np.float32)),
        "wot": np.ascontiguousarray(wo.T.astype(np.float32)),
        "bq": (bq * s).astype(np.float32),
        "bk": bk.astype(np.float32),
        "bvrow": bv.astype(np.float32).reshape(1, C),
        "gamma": gn_gamma.astype(np.float32),
        "beta": gn_beta.astype(np.float32),
        "gmask": np.repeat(np.eye(GPC, dtype=np.float32), 16, axis=0) / 16.0,
        "gbcast": np.repeat(np.eye(GPC, dtype=np.float32), 16, axis=1),
        "ones128": np.ones((P, 1), np.float32),
    }
    in_maps = []
    for core in range(8):
        b, half = core // 2, core % 2
        xb = np.asarray(x[b], np.float32).reshape(C, N)
        xrot = np.roll(xb, -half * NQ, axis=1)
        in_maps.append({
            **shared,
            "x": np.ascontiguousarray(xrot),
            "xr": np.ascontiguousarray(xb[:, half * NQ:(half + 1) * NQ]
                                       + bo[:, None].astype(np.float32)),
        })
    return in_maps


_NC_CACHE = {}


def kernel(**inputs):
    inputs = {k: np.asarray(v) for k, v in inputs.items()}
    if "nc" not in _NC_CACHE:
        _NC_CACHE["nc"] = build_nc()
    nc = _NC_CACHE["nc"]
    in_maps = _host_inputs(**inputs)
    res = run_bass_kernel_spmd(nc, in_maps, core_ids=list(range(8)))
    out = np.empty((B, C, N), np.float32)
    for core in range(8):
        b, half = core // 2, core % 2
        out[b, :, half * NQ:(half + 1) * NQ] = res.results[core]["y"]
    return out.reshape(B, C, H, W)


# revision 24
# speedup vs baseline: 2.3797x; 2.3797x over previous
"""AttnBlock (GroupNorm + single-head self-attention + residual) on 8 TRN2 cores.

Problem: x [4, 512, 64, 64] f32.  out = x + proj_out(attn(GN(x))) with
1x1-conv projections and softmax attention over the 4096 spatial positions.

Sharding: 8 cores = 4 batch elements x 2 query-halves.  Each core gets the
full (column-rotated) [512, 4096] slab of its batch element, computes
GroupNorm + K/V projections over all 4096 positions, and Q/attention/output
for its 2048 query columns.  Column rotation makes the program SPMD: every
core's queries are columns 0:2048 of its own input.  No collectives; the
host gathers the 8 [512, 2048] results.

All heavy matmuls run as float32r (TF32-like, full PE rate at N=512,
~1.5e-4 matmul rel-err).  GroupNorm stats and softmax accumulate in f32.
Softmax: scores^T tiles [keys,128 x q,512] so QK^T and AV need no
transposes; row sums via ones-vector matmul; no max subtraction (logits are
~N(0,1), |logit| << 80).
"""
import numpy as np

import concourse.bacc as bacc
import concourse.tile as tile
import concourse.mybir as mybir
from concourse.bass_utils import run_bass_kernel_spmd

F32 = mybir.dt.float32
F32R = mybir.dt.float32r
AF = mybir.ActivationFunctionType
ALU = mybir.AluOpType

B, C, H, W = 4, 512, 64, 64
N = H * W           # 4096 key positions
NQ = N // 2         # 2048 query positions per core
P = 128             # partitions
CCH = C // P        # 4 channel chunks
QS = 512            # query-chunk width
NQCH = NQ // QS     # 4 query chunks
MCH = N // P        # 32 key tiles
NSUB = 8            # bn_stats subtiles (4096 / 512)
NUM_GROUPS = 32
GPC = P // (C // NUM_GROUPS)   # groups per 128-channel chunk = 8
EPS = 1e-6


OPTS = {"stagger": True, "defer": True, "act_norm": True, "psq_share": True, "split_kv": True, "kcopy_dve": False, "qfuse": True, "ps2_pso": True, "ea4": True, "fold": True}


def build_nc(reps: int = 1, opts=None):
    o_ = dict(OPTS)
    if opts:
        o_.update(opts)
    nc = bacc.Bacc()
    x_d = nc.dram_tensor("x", [C, N], F32, kind="ExternalInput")
    wqt_d = nc.dram_tensor("wqt", [C, C], F32, kind="ExternalInput")
    wkt_d = nc.dram_tensor("wkt", [C, C], F32, kind="ExternalInput")
    wvt_d = nc.dram_tensor("wvt", [C, C], F32, kind="ExternalInput")
    wot_d = nc.dram_tensor("wot", [C, C], F32, kind="ExternalInput")
    bq_d = nc.dram_tensor("bq", [C], F32, kind="ExternalInput")
    bk_d = nc.dram_tensor("bk", [C], F32, kind="ExternalInput")
    bvrow_d = nc.dram_tensor("bvrow", [1, C], F32, kind="ExternalInput")
    gamma_d = nc.dram_tensor("gamma", [C], F32, kind="ExternalInput")
    beta_d = nc.dram_tensor("beta", [C], F32, kind="ExternalInput")
    xr_d = nc.dram_tensor("xr", [C, NQ], F32, kind="ExternalInput")
    gmask_d = nc.dram_tensor("gmask", [P, GPC], F32, kind="ExternalInput")
    gbcast_d = nc.dram_tensor("gbcast", [GPC, P], F32, kind="ExternalInput")
    ones128_d = nc.dram_tensor("ones128", [P, 1], F32, kind="ExternalInput")
    y_d = nc.dram_tensor("y", [C, NQ], F32, kind="ExternalOutput")

    x_t = x_d.rearrange("(c p) n -> p c n", p=P)
    xr_t = xr_d.rearrange("(c p) n -> p c n", p=P)
    y_t = y_d.rearrange("(c p) n -> p c n", p=P)

    with tile.TileContext(nc) as tc:
        def body(_iv=None):
            with tc.tile_pool(name="consts", bufs=1) as consts, \
                 tc.tile_pool(name="dram", bufs=1, space="DRAM") as dpool:
                hd = None
                if not o_["fold"]:
                    hd = dpool.tile([P, CCH, N], F32R, tag="hd")

                gam = consts.tile([P, CCH], F32, tag="gam")
                bet = consts.tile([P, CCH], F32, tag="bet")
                gmask = consts.tile([P, GPC], F32, tag="gmask")
                gbcast = consts.tile([GPC, P], F32, tag="gbcast")
                ones128 = consts.tile([P, 1], F32R, tag="ones128")
                bvrow = consts.tile([1, C], F32, tag="bvrow")
                bvb = consts.tile([P, C], F32, tag="bvb")
                bqs = consts.tile([P, CCH], F32, tag="bqs")
                bks = consts.tile([P, CCH], F32, tag="bks")
                epst = consts.tile([GPC, 1], F32, tag="epst")
                nc.sync.dma_start(out=gam, in_=gamma_d.rearrange("(c p) -> p c", p=P))
                nc.sync.dma_start(out=bet, in_=beta_d.rearrange("(c p) -> p c", p=P))
                nc.sync.dma_start(out=gmask, in_=gmask_d[:, :])
                nc.sync.dma_start(out=gbcast, in_=gbcast_d[:, :])
                nc.gpsimd.dma_start(out=ones128, in_=ones128_d[:, :])
                nc.sync.dma_start(out=bvrow, in_=bvrow_d[:, :])
                if not o_["fold"]:
                    nc.gpsimd.partition_broadcast(bvb, bvrow)
                nc.sync.dma_start(out=bqs, in_=bq_d.rearrange("(c p) -> p c", p=P))
                nc.sync.dma_start(out=bks, in_=bk_d.rearrange("(c p) -> p c", p=P))
                nc.vector.memset(epst, EPS)

                # ---------------- Phase A: GroupNorm -> h (DRAM, f32r) ----
                with tc.tile_pool(name="pA", bufs=1) as pA, \
                     tc.tile_pool(name="pAs", bufs=2) as pAs, \
                     tc.tile_pool(name="pAh", bufs=3) as pAh, \
                     tc.tile_pool(name="psA", bufs=2, space="PSUM") as psA:
                    xt = pA.tile([P, CCH, N], F32, tag="xt")
                    for c in range(CCH):
                        for s in range(NSUB):
                            nc.sync.dma_start(
                                out=xt[:, c, s * QS:(s + 1) * QS],
                                in_=x_t[:, c, s * QS:(s + 1) * QS])

                    stats = pA.tile([P, CCH, NSUB, 6], F32, tag="stats")
                    mv = pA.tile([P, CCH, 2], F32, tag="mv")
                    st3 = pA.tile([P, CCH, 3], F32, tag="st3")
                    scb = consts.tile([P, CCH, 2], F32, tag="scb")
                    for c in range(CCH):
                        for s in range(NSUB):
                            nc.vector.bn_stats(
                                out=stats[:, c, s, :],
                                in_=xt[:, c, s * QS:(s + 1) * QS])
                        nc.vector.bn_aggr(out=mv[:, c, :], in_=stats[:, c, :, :])
                        nc.vector.tensor_copy(st3[:, c, 0:2], mv[:, c, :])
                        nc.vector.tensor_mul(
                            st3[:, c, 2:3], mv[:, c, 0:1], mv[:, c, 0:1])
                    for c in range(CCH):
                        psg = psA.tile([GPC, 3], F32, tag="psg")
                        nc.tensor.matmul(psg, gmask, st3[:, c, :],
                                         start=True, stop=True)
                        sg = pAs.tile([GPC, 3], F32, tag="sg")
                        nc.scalar.activation(sg, psg, AF.Copy)
                        gst = pAs.tile([GPC, 4], F32, tag="gst")
                        # var_g = E[var] + E[mean^2] - (E[mean])^2
                        nc.vector.tensor_add(gst[:, 1:2], sg[:, 1:2], sg[:, 2:3])
                        nc.vector.tensor_mul(gst[:, 2:3], sg[:, 0:1], sg[:, 0:1])
                        nc.vector.tensor_tensor(
                            out=gst[:, 1:2], in0=gst[:, 1:2], in1=gst[:, 2:3],
                            op=ALU.subtract)
                        nc.scalar.activation(gst[:, 3:4], gst[:, 1:2], AF.Sqrt,
                                             bias=epst)
                        nc.vector.reciprocal(gst[:, 3:4], gst[:, 3:4])
                        gs2 = pAs.tile([GPC, 2], F32, tag="gs2")
                        nc.vector.tensor_copy(gs2[:, 0:1], sg[:, 0:1])
                        nc.vector.tensor_copy(gs2[:, 1:2], gst[:, 3:4])
                        psb = psA.tile([P, 2], F32, tag="psb")
                        nc.tensor.matmul(psb, gbcast, gs2, start=True, stop=True)
                        # scale_c = gamma*rstd ; bias_c = beta - mean*scale
                        nc.vector.tensor_mul(
                            scb[:, c, 0:1], gam[:, c:c + 1], psb[:, 1:2])
                        tmp = pAs.tile([P, 1], F32, tag="tmp")
                        nc.vector.tensor_mul(tmp, psb[:, 0:1], scb[:, c, 0:1])
                        nc.vector.tensor_tensor(
                            out=scb[:, c, 1:2], in0=bet[:, c:c + 1], in1=tmp,
                            op=ALU.subtract)
                    if o_["fold"]:
                        bias_r = consts.tile([P, CCH, 4], F32R, tag="bias_r")
                        for j in range(4):
                            nc.vector.tensor_copy(bias_r[:, :, j], scb[:, :, 1])
                    for s in range(NSUB if not o_["fold"] else 0):
                        for c in range(CCH):
                            ht = pAh.tile([P, QS], F32R, tag="ht")
                            if o_["act_norm"]:
                                nc.scalar.activation(
                                    ht, xt[:, c, s * QS:(s + 1) * QS],
                                    AF.Identity,
                                    bias=scb[:, c, 1:2], scale=scb[:, c, 0:1])
                            else:
                                nc.vector.tensor_scalar(
                                    out=ht, in0=xt[:, c, s * QS:(s + 1) * QS],
                                    scalar1=scb[:, c, 0:1],
                                    scalar2=scb[:, c, 1:2],
                                    op0=ALU.mult, op1=ALU.add)
                            nc.sync.dma_start(
                                out=hd[:, c, s * QS:(s + 1) * QS], in_=ht)

                # ---------------- persistent K / V^T ----------------------
                with tc.tile_pool(name="pKV", bufs=1) as pKV:
                    if o_["qfuse"]:
                        qts = [pKV.tile([P, CCH, QS], F32R, tag=f"qts{_s}",
                                        name=f"qts{_s}")
                               for _s in range(NQCH)]
                    if o_["split_kv"]:
                        ks = [pKV.tile([P, CCH, QS], F32R, tag=f"ks{_s}",
                                       name=f"ks{_s}")
                              for _s in range(NSUB)]
                        vts = [pKV.tile([P, CCH, C], F32R, tag=f"vts{_s}",
                                        name=f"vts{_s}")
                               for _s in range(NSUB)]
                    else:
                        k_one = pKV.tile([P, CCH, N], F32R, tag="k_sb")
                        vt_one = pKV.tile([P, NSUB, CCH, C], F32R,
                                          tag="vt_sb")
                        ks = [k_one[:, :, _s * QS:(_s + 1) * QS]
                              for _s in range(NSUB)]
                        vts = [vt_one[:, _s] for _s in range(NSUB)]

                    # -------- Phase B: K and V^T projections --------------
                    with tc.tile_pool(name="pBw", bufs=1) as pBw, \
                         tc.tile_pool(name="pBh",
                                      bufs=o_.get("pbh", 2)
                                      if o_["qfuse"] else 3) as pBh, \
                         tc.tile_pool(name="psB",
                                      bufs=2 if o_["qfuse"] else 4,
                                      space="PSUM") as psB:
                        wkt = pBw.tile([P, CCH, C], F32R, tag="wkt")
                        wvt = pBw.tile([P, CCH, C], F32R, tag="wvt")
                        nc.gpsimd.dma_start(
                            out=wkt, in_=wkt_d.rearrange("(c p) o -> p c o", p=P))
                        nc.gpsimd.dma_start(
                            out=wvt, in_=wvt_d.rearrange("(c p) o -> p c o", p=P))
                        if o_["qfuse"]:
                            wqt_b = pBw.tile([P, CCH, C], F32R, tag="wqt_b")
                            nc.gpsimd.dma_start(
                                out=wqt_b,
                                in_=wqt_d.rearrange("(c p) o -> p c o", p=P))
                        biask = bks
                        biasq = bqs
                        if o_["fold"]:
                            # h = S.x + b folds into the projections:
                            #   k = (wk.S).x + (wk.b + bk)  etc.
                            # bias matvecs first (need unscaled weights)
                            with tc.tile_pool(name="psBb", bufs=1,
                                              space="PSUM") as psBb:
                                kqb = pBw.tile([P, CCH, 2], F32, tag="kqb")
                                for o in range(CCH):
                                    for j, wt in enumerate((wkt, wqt_b)):
                                        psbb = psBb.tile(
                                            [P, 4], F32, tag="psbb",
                                            name=f"psbb{o}_{j}")
                                        for c in range(CCH):
                                            nc.tensor.matmul(
                                                psbb,
                                                wt[:, c, o * P:(o + 1) * P],
                                                bias_r[:, c, :],
                                                start=(c == 0),
                                                stop=(c == CCH - 1))
                                        nc.scalar.activation(
                                            kqb[:, o, j:j + 1],
                                            psbb[:, 0:1], AF.Copy)
                                psvb = psBb.tile([1, C], F32, tag="psvb")
                                for c in range(CCH):
                                    nc.tensor.matmul(
                                        psvb, bias_r[:, c, 0:1], wvt[:, c, :],
                                        start=(c == 0), stop=(c == CCH - 1))
                                vbrow = pBw.tile([1, C], F32, tag="vbrow")
                                nc.vector.tensor_add(vbrow, psvb, bvrow)
                                nc.gpsimd.partition_broadcast(bvb, vbrow)
                            biask = pBw.tile([P, CCH], F32, tag="biask")
                            biasq = pBw.tile([P, CCH], F32, tag="biasq")
                            nc.vector.tensor_add(biask, kqb[:, :, 0], bks)
                            nc.vector.tensor_add(biasq, kqb[:, :, 1], bqs)
                            # scale weight rows in place (f32r -> f32r)
                            for c in range(CCH):
                                for wt in (wkt, wvt, wqt_b):
                                    nc.vector.tensor_scalar_mul(
                                        out=wt[:, c, :], in0=wt[:, c, :],
                                        scalar1=scb[:, c, 0:1])
                        for s in range(NSUB):
                            ht = pBh.tile([P, CCH, QS], F32R, tag="htb")
                            if o_["fold"]:
                                nc.gpsimd.dma_start(
                                    out=ht,
                                    in_=x_t[:, :, s * QS:(s + 1) * QS])
                            else:
                                nc.sync.dma_start(
                                    out=ht,
                                    in_=hd[:, :, s * QS:(s + 1) * QS])
                            for o in range(CCH):
                                psk = psB.tile([P, QS], F32, tag="psk")
                                for c in range(CCH):
                                    nc.tensor.matmul(
                                        psk, wkt[:, c, o * P:(o + 1) * P],
                                        ht[:, c, :],
                                        start=(c == 0), stop=(c == CCH - 1))
                                if o_["kcopy_dve"]:
                                    nc.vector.tensor_scalar(
                                        out=ks[s][:, o, :], in0=psk,
                                        scalar1=biask[:, o:o + 1],
                                        scalar2=None, op0=ALU.add)
                                else:
                                    nc.scalar.activation(
                                        ks[s][:, o, :], psk,
                                        AF.Identity, bias=biask[:, o:o + 1])
                            for mm in range(CCH):
                                m = s * CCH + mm
                                psv = psB.tile([P, C], F32, tag="psv")
                                for c in range(CCH):
                                    nc.tensor.matmul(
                                        psv, ht[:, c, mm * P:(mm + 1) * P],
                                        wvt[:, c, :],
                                        start=(c == 0), stop=(c == CCH - 1))
                                nc.vector.tensor_add(vts[s][:, mm, :], psv,
                                                     bvb)
                            if o_["qfuse"] and s < NQCH:
                                for o in range(CCH):
                                    psq = psB.tile([P, QS], F32, tag="psq",
                                                   name=f"psqB{s}_{o}")
                                    for c in range(CCH):
                                        nc.tensor.matmul(
                                            psq,
                                            wqt_b[:, c, o * P:(o + 1) * P],
                                            ht[:, c, :],
                                            start=(c == 0),
                                            stop=(c == CCH - 1))
                                    nc.scalar.activation(
                                        qts[s][:, o, :], psq, AF.Identity,
                                        bias=biasq[:, o:o + 1])

                    # -------- Phase C: Q, attention, output ---------------
                    with tc.tile_pool(name="pCw", bufs=1) as pCw, \
                         tc.tile_pool(name="pCh", bufs=1) as pCh, \
                         tc.tile_pool(name="pCq", bufs=2) as pCq, \
                         tc.tile_pool(name="pCo", bufs=1) as pCo, \
                         tc.tile_pool(name="pCe",
                                      bufs=4 if o_["ea4"] else 3) as pCe, \
                         tc.tile_pool(name="pCs", bufs=1) as pCs, \
                         tc.tile_pool(name="pCy", bufs=2) as pCy, \
                         tc.tile_pool(name="psO", bufs=4, space="PSUM") as psO_p, \
                         tc.tile_pool(name="psS", bufs=1, space="PSUM") as psS_p, \
                         tc.tile_pool(name="psAtt",
                                      bufs=3 if o_["psq_share"] else 2,
                                      space="PSUM") as psAtt, \
                         tc.tile_pool(name="psMM", bufs=1,
                                      space="PSUM") as psMM:
                        wot = pCw.tile([P, CCH, C], F32R, tag="wot")
                        nc.gpsimd.dma_start(
                            out=wot, in_=wot_d.rearrange("(c p) o -> p c o", p=P))
                        if not o_["qfuse"]:
                            wqt = pCw.tile([P, CCH, C], F32R, tag="wqt")
                            nc.gpsimd.dma_start(
                                out=wqt,
                                in_=wqt_d.rearrange("(c p) o -> p c o", p=P))

                        for iq in range(NQCH):
                            qsl = slice(iq * QS, (iq + 1) * QS)
                            if o_["qfuse"]:
                                qt = qts[iq]
                            else:
                                hq = pCh.tile([P, CCH, QS], F32R, tag="hq")
                                nc.sync.dma_start(out=hq, in_=hd[:, :, qsl])
                                qt = pCq.tile([P, CCH, QS], F32R, tag="qt")
                            for o in range(CCH if not o_["qfuse"] else 0):
                                if o_["psq_share"]:
                                    psq = psAtt.tile([P, QS], F32, tag="psa",
                                                     name=f"psq{o}")
                                else:
                                    psq = psMM.tile([P, QS], F32, tag="psmm",
                                                    name=f"psq{o}")
                                for c in range(CCH):
                                    nc.tensor.matmul(
                                        psq, wqt[:, c, o * P:(o + 1) * P],
                                        hq[:, c, :],
                                        start=(c == 0), stop=(c == CCH - 1))
                                if o_["kcopy_dve"]:
                                    nc.vector.tensor_scalar(
                                        out=qt[:, o, :], in0=psq,
                                        scalar1=bqs[:, o:o + 1], scalar2=None,
                                        op0=ALU.add)
                                else:
                                    nc.scalar.activation(
                                        qt[:, o, :], psq, AF.Identity,
                                        bias=bqs[:, o:o + 1])

                            psO = [psO_p.tile([P, QS], F32, tag="psO",
                                              name=f"psO{_c}")
                                   for _c in range(CCH)]
                            pssum = psS_p.tile([1, QS], F32, tag="pssum")
                            # software-pipelined: QK(m) runs ahead of AV(m-1)
                            # so the PE never waits on exp(m-1) (ACT).
                            ea_prev = None

                            def qk(m):
                                psa = psAtt.tile([P, QS], F32, tag="psa",
                                                 name=f"psa{m}")
                                for c in range(CCH):
                                    nc.tensor.matmul(
                                        psa,
                                        ks[m // CCH][:, c,
                                                     (m % CCH) * P:
                                                     (m % CCH + 1) * P],
                                        qt[:, c, :],
                                        start=(c == 0), stop=(c == CCH - 1))
                                ea = pCe.tile([P, QS], F32R, tag="ea",
                                              name=f"ea{m}")
                                nc.scalar.activation(ea, psa, AF.Exp)
                                return ea

                            def av(m, ea):
                                for c in range(CCH):
                                    nc.tensor.matmul(
                                        psO[c],
                                        vts[m // CCH][:, m % CCH,
                                                      c * P:(c + 1) * P], ea,
                                        start=(m == 0), stop=(m == MCH - 1),
                                        skip_group_check=True)
                                nc.tensor.matmul(
                                    pssum, ones128, ea,
                                    start=(m == 0), stop=(m == MCH - 1),
                                    skip_group_check=True)

                            if o_["stagger"]:
                                ea_prev = qk(0)
                                for m in range(1, MCH):
                                    ea_cur = qk(m)
                                    av(m - 1, ea_prev)
                                    ea_prev = ea_cur
                                av(MCH - 1, ea_prev)
                            else:
                                for m in range(MCH):
                                    av(m, qk(m))

                            rec = pCs.tile([1, QS], F32, tag="rec")
                            nc.vector.reciprocal(rec, pssum)
                            recb = pCs.tile([P, QS], F32, tag="recb")
                            nc.gpsimd.partition_broadcast(recb, rec)
                            on = pCo.tile([P, CCH, QS], F32R, tag="on")
                            if o_["defer"]:
                                # unnormalized O^T -> SBUF; out2 starts
                                # immediately; 1/rowsum applied at the end.
                                for c in range(CCH):
                                    nc.scalar.activation(on[:, c, :], psO[c],
                                                         AF.Copy)
                            else:
                                for c in range(CCH):
                                    nc.vector.tensor_mul(on[:, c, :], psO[c],
                                                         recb)
                            xrt = pCo.tile([P, CCH, QS], F32, tag="xrt")
                            nc.sync.dma_start(out=xrt, in_=xr_t[:, :, qsl])
                            for o in range(CCH):
                                if o_["ps2_pso"]:
                                    ps2 = psO_p.tile([P, QS], F32, tag="psO",
                                                     name=f"ps2{o}")
                                elif o_["psq_share"]:
                                    ps2 = psAtt.tile([P, QS], F32, tag="psa",
                                                     name=f"ps2{o}")
                                else:
                                    ps2 = psMM.tile([P, QS], F32, tag="psmm",
                                                    name=f"ps2{o}")
                                for c in range(CCH):
                                    nc.tensor.matmul(
                                        ps2, wot[:, c, o * P:(o + 1) * P],
                                        on[:, c, :],
                                        start=(c == 0), stop=(c == CCH - 1))
                                yt = pCy.tile([P, QS], F32, tag="yt")
                                if o_["defer"]:
                                    t1 = pCs.tile([P, QS], F32, tag="t1")
                                    nc.vector.tensor_mul(t1, ps2, recb)
                                    nc.vector.tensor_add(yt, t1, xrt[:, o, :])
                                else:
                                    nc.vector.tensor_add(yt, ps2, xrt[:, o, :])
                                nc.sync.dma_start(out=y_t[:, o, qsl], in_=yt)

        if reps == 1:
            body()
        else:
            with tc.For_i(0, reps, 1) as iv:
                body(iv)
    nc.compile()
    return nc


def _host_inputs(x, gn_gamma, gn_beta, wq, bq, wk, bk, wv, bv, wo, bo):
    """Build the 8 per-core input maps from full inputs."""
    s = 1.0 / np.sqrt(np.float32(C))
    shared = {
        "wqt": np.ascontiguousarray((wq * s).T.astype(np.float32)),
        "wkt": np.ascontiguousarray(wk.T.astype(np.float32)),
        "wvt": np.ascontiguousarray(wv.T.astype(np.float32)),
        "wot": np.ascontiguousarray(wo.T.astype(np.float32)),
        "bq": (bq * s).astype(np.float32),
        "bk": bk.astype(np.float32),
        "bvrow": bv.astype(np.float32).reshape(1, C),
        "gamma": gn_gamma.astype(np.float32),
        "beta": gn_beta.astype(np.float32),
        "gmask": np.repeat(np.eye(GPC, dtype=np.float32), 16, axis=0) / 16.0,
        "gbcast": np.repeat(np.eye(GPC, dtype=np.float32), 16, axis=1),
        "ones128": np.ones((P, 1), np.float32),
    }
    in_maps = []
    for core in range(8):
        b, half = core // 2, core % 2
        xb = np.asarray(x[b], np.float32).reshape(C, N)
        xrot = np.roll(xb, -half * NQ, axis=1)
        in_maps.append({
            **shared,
            "x": np.ascontiguousarray(xrot),
            "xr": np.ascontiguousarray(xb[:, half * NQ:(half + 1) * NQ]
                                       + bo[:, None].astype(np.float32)),
        })
    return in_maps


_NC_CACHE = {}


def kernel(**inputs):
    inputs = {k: np.asarray(v) for k, v in inputs.items()}
    if "nc" not in _NC_CACHE:
        _NC_CACHE["nc"] = build_nc()
    nc = _NC_CACHE["nc"]
    in_maps = _host_inputs(**inputs)
    res = run_bass_kernel_spmd(nc, in_maps, core_ids=list(range(8)))
    out = np.empty((B, C, N), np.float32)
    for core in range(8):
        b, half = core // 2, core % 2
        out[b, :, half * NQ:(half + 1) * NQ] = res.results[core]["y"]
    return out.reshape(B, C, H, W)
